# revision 28
# baseline (speedup 1.0000x reference)
"""GAT layer kernel for Trainium2 (8 NeuronCores, SPMD data-parallel over B).

Reference computation (per (b,t) slice, N=512 nodes, D=F=128):
    h = x_bt @ W
    e[i,j] = leaky_relu(e_src[i] + e_dst[j], 0.2)
    e masked by adj|I, row-softmax, out = elu(alpha @ h)

v6 dataflow. Key identity: exp(leaky(e, 0.2)) = exp(0.2e) * max(exp(0.8e), 1)
 = C_i * D_j * max(u_ji, 1) with u = exp(0.8(es_i + ed_j)), C = exp(0.2 es),
D = exp(0.2 ed). The per-row factor C_i cancels in the softmax (alpha = z/s),
so the kernel aggregates z2_ji = D_j * max(u_ji, 1) * m_ij instead of the
full exp(leaky): ONE big activation pass (exp, scale=0.8) replaces the
baseline's Prelu+Exp pair, and the adjacency mask is applied
multiplicatively (no PE mask-prefill matmul). 2-byte tensors are fp16
(u <= exp(8.8) ~ 6.6e3 fits; fp16 mantissa keeps rel err ~7e-4).

Per (b,t), in eT = e^T [j, i] orientation so aggregation runs as
PSUM-accumulated matmuls with j as the contraction dim. Four software-
pipelined stages (A prep, B z2 production, C aggregation, D tail) with
lags (2,1,1), emitted oldest-first ("dbca") so no in-order engine queue
head-blocks on not-yet-ready work:
  A: ev rows [ed; es] = [wd|ws].T @ xT -> f32 PSUM; DVE evac to f16;
     ev_lhs=[ed;1], ev_rhs=[1;es] via Pool tensor_scalar (sel ptrs);
     D_j = exp(0.2 ed): 4 PE column-transposes (4B-aligned f16 slots)
     + one tiny ACT exp; h = xT @ W -> f32 PSUM -> DVE evac f16
  B: rank-2 matmul per chunk -> f32 PSUM (ring3); ACT exp(0.8 e) -> f16
     SBUF (the only big ACT pass); z2 = max(u*D, D) in-place DVE ts
     (4x mode, per-partition D ptr); mask: in-place DVE tt *m01 in
     halves (2x mode)
  C: s = ones.T @ z2, uT = h.T @ z2 (PSUM-accumulated); s_row via ACT
     copy, uT via ACT copy (f16); PE-transpose s to columns; DVE
     reciprocal_approx_fast [128,4]
  D: PE-transpose uT back to [i, f] f16 PSUM; v = u * r (DVE ts, f32);
     elu(v) = max(v,0) + min(exp(v)-1, 0): ACT exp, Pool e1/m ts,
     Pool add (gpsimd TT supports add/mult only, f32 only for TT)
"""

import numpy as np

B, N, T, D, F = 16, 512, 12, 128, 128
NCORES = 8
B_PER_CORE = B // NCORES
NCH = N // 128  # 4 chunks of 128 nodes


def _build_program(reps=1):
    import concourse.bacc as bacc
    import concourse.tile as tile
    from concourse import mybir

    import os
    F32 = mybir.dt.float32
    F16 = mybir.dt.float16
    AF = mybir.ActivationFunctionType
    ALU = mybir.AluOpType

    nc = bacc.Bacc()

    def eng(name):
        return {"pool": nc.gpsimd, "vector": nc.vector, "act": nc.scalar}[name]

    # engine assignment knobs
    E_HCOPY = os.environ.get("K_HCOPY", "vector")   # h evac: vector|act
    E_UTCOPY = os.environ.get("K_UTCOPY", "act")    # uT evac: vector|act
    E_SROW = os.environ.get("K_SROW", "act")        # s_row evac: vector|act
    E_FIX = os.environ.get("K_FIX", "pool")         # ev fixups: pool|vector
    E_E1 = os.environ.get("K_E1", "pool")         # tail e1: pool|vector
    E_OMAX = os.environ.get("K_OMAX", "pool")       # tail o=max: pool|vector
    MSPLIT = int(os.environ.get("K_MSPLIT", "2"))   # z2 mask-mult pieces

    x_h = nc.declare_dram_parameter("x", [B_PER_CORE, N, T, D], F16, isOutput=False)
    wb_h = nc.declare_dram_parameter("wb", [D, F], F16, isOutput=False)
    wsd_h = nc.declare_dram_parameter("wsd", [D, 2], F16, isOutput=False)
    capt_h = nc.declare_dram_parameter("capt", [NCH, 128, N], F16, isOutput=False)
    sel_h = nc.declare_dram_parameter("sel", [2, 2], F32, isOutput=False)
    ident_h = nc.declare_dram_parameter("ident", [128, 128], F32, isOutput=False)
    identh_h = nc.declare_dram_parameter("identh", [128, 128], F16, isOutput=False)
    ones_h = nc.declare_dram_parameter("onescol", [128, 1], F16, isOutput=False)
    out_h = nc.declare_dram_parameter("out", [B_PER_CORE, N, T, F], F32, isOutput=True)

    NBT = B_PER_CORE * T

    with tile.TileContext(nc) as tc:
        with (
            tc.tile_pool(name="consts", bufs=1) as consts,
            tc.tile_pool(name="xbuf", bufs=1) as xbuf,
            tc.tile_pool(name="work", bufs=int(os.environ.get("K_WORK", "5"))) as work,
            tc.tile_pool(name="zpool", bufs=int(os.environ.get("K_BIG", "5"))) as zpool,
            tc.tile_pool(name="hpool", bufs=int(os.environ.get("K_HP", "5"))) as hpool,
            # PSUM (16KB/partition = 8 banks of 2KB):
            #   mm ring3 x 2KB = 3, eadd (f32 quarters, 2KB) ring3 = 3,
            #   ut (f32 [128,512]) ring2 = 2  -> 8 banks
            tc.tile_pool(name="mm_ps", bufs=int(os.environ.get("K_MM", "3")), space="PSUM") as mm_ps,
            tc.tile_pool(name="eadd_ps", bufs=int(os.environ.get("K_EADD", "3")), space="PSUM") as eadd_ps,
            tc.tile_pool(name="ut_ps", bufs=int(os.environ.get("K_UT", "2")), space="PSUM") as ut_psp,
        ):
            wb_sb = consts.tile([D, F], F16)
            wsd_sb = consts.tile([D, 2], F16)
            cap_sb = consts.tile([128, NCH, N], F16)
            sel_sb = consts.tile([2, 2], F32)
            id_sb = consts.tile([128, 128], F32)
            idh_sb = consts.tile([128, 128], F16)
            ones_sb = consts.tile([128, 1], F16)

            # ---- transpose-DMA all of x: [n, d] slices land as [d, n] f16.
            XCH = int(os.environ.get("K_XCH", "4"))  # chunks per x-DMA
            XPRI = int(os.environ.get("K_XPRI", "2"))  # bts loaded pre-consts
            xT_all = xbuf.tile([128, NBT, N], F16, tag="xT")

            def load_x(k):
                b, t = divmod(k, T)
                for c0 in range(0, NCH, XCH):
                    c1 = min(c0 + XCH, NCH)
                    nc.sync.dma_start_transpose(
                        out=xT_all[:, k, c0 * 128:c1 * 128],
                        in_=x_h[b, c0 * 128:c1 * 128, t, :])

            # DMA order follows first use. x-loads go on the sync (SP)
            # queue; consts are issued in parallel from the Pool DGE queue
            # (idle at startup) so neither serializes the other. The big
            # cap_sb (mask) load goes last among early consts -- first use
            # is stageB's mask-mult, well after ev/dcol/h.
            cq = nc.sync
            if os.environ.get("K_DGE", "0") == "1":
                # consts issued from the (startup-idle) ACT/DVE DGE queues
                # so the SP queue only carries x-loads; everything lands
                # earlier and x(1..) isn't stuck behind the big cap issue.
                nc.scalar.dma_start(out=wsd_sb, in_=wsd_h[:, :])
                nc.gpsimd.dma_start(out=sel_sb, in_=sel_h[:, :])
                nc.gpsimd.dma_start(out=idh_sb, in_=identh_h[:, :])
                nc.scalar.dma_start(
                    out=cap_sb,
                    in_=capt_h[:, :, :].rearrange("c p i -> p c i"))
                nc.gpsimd.dma_start(out=wb_sb, in_=wb_h[:, :])
                nc.gpsimd.dma_start(out=ones_sb, in_=ones_h[:, :])
                nc.gpsimd.dma_start(out=id_sb, in_=ident_h[:, :])
                for k in range(NBT):
                    load_x(k)
            else:
                for k in range(min(XPRI, NBT)):
                    load_x(k)
                cq.dma_start(out=wsd_sb, in_=wsd_h[:, :])
                cq.dma_start(out=sel_sb, in_=sel_h[:, :])
                cq.dma_start(out=idh_sb, in_=identh_h[:, :])
                cq.dma_start(
                    out=cap_sb, in_=capt_h[:, :, :].rearrange("c p i -> p c i"))
                cq.dma_start(out=wb_sb, in_=wb_h[:, :])
                XDEF = int(os.environ.get("K_XDEF", "4"))
                for k in range(min(XPRI, NBT), min(XDEF, NBT)):
                    load_x(k)
                cq.dma_start(out=ones_sb, in_=ones_h[:, :])
                cq.dma_start(out=id_sb, in_=ident_h[:, :])
                for k in range(min(XDEF, NBT), NBT):
                    load_x(k)

            st = [dict() for _ in range(NBT)]

            def stageA(k):
                """prep: ev matmul+evac, D columns, h matmul+evac"""
                xT = xT_all[:, k, :]
                # ev rows [ed; es] (f32 PSUM), evac, fixups
                ev_ps = mm_ps.tile([2, N], F32, tag="mm")
                nc.tensor.matmul(ev_ps, wsd_sb, xT, start=True, stop=True)
                evb_sb = work.tile([2, N], F16, tag="evb")
                nc.vector.tensor_copy(out=evb_sb, in_=ev_ps)
                ev_rhs = work.tile([2, N], F16, tag="ev_rhs")
                ev_lhs = work.tile([2, N], F16, tag="ev_lhs")
                eng(E_FIX).tensor_scalar(
                    out=ev_lhs, in0=evb_sb, scalar1=sel_sb[:, 0:1],
                    scalar2=sel_sb[:, 1:2], op0=ALU.mult, op1=ALU.add)
                eng(E_FIX).tensor_scalar(
                    out=ev_rhs, in0=evb_sb, scalar1=sel_sb[:, 1:2],
                    scalar2=sel_sb[:, 0:1], op0=ALU.mult, op1=ALU.add)

                # D_j = exp(0.2 ed_j) as columns [128, NCH]. The f16
                # transpose outputs go to even column slots so each PSUM
                # write is 4-byte aligned (verifier requirement).
                dcol_ps = ut_psp.tile([128, 2 * NCH], F16, tag="ut")
                for c in range(NCH):
                    nc.tensor.transpose(
                        dcol_ps[:, 2 * c:2 * c + 1],
                        evb_sb[0:1, c * 128:(c + 1) * 128], idh_sb[0:1, 0:1])
                dcol_sb = work.tile([128, NCH], F32, tag="dcol")
                nc.scalar.activation(
                    dcol_sb, dcol_ps[:, 0:2 * NCH:2], AF.Exp, scale=0.2)

                # h projection -> f32 PSUM -> f16 SBUF
                h_ps = mm_ps.tile([128, NCH, F], F32, tag="mm")
                for c in range(NCH):
                    nc.tensor.matmul(
                        h_ps[:, c, :], xT[:, c * 128:(c + 1) * 128],
                        wb_sb, start=True, stop=True)
                h_sb = hpool.tile([128, NCH, F], F16, tag="h_sb")
                if E_HCOPY == "act":
                    nc.scalar.copy(out=h_sb, in_=h_ps)
                else:
                    nc.vector.tensor_copy(out=h_sb, in_=h_ps)
                st[k]["h_sb"] = h_sb
                st[k]["ev_rhs"] = ev_rhs
                st[k]["ev_lhs"] = ev_lhs
                st[k]["dcol_sb"] = dcol_sb
                z_sb = zpool.tile([128, NCH, N], F16, tag="z_sb")
                st[k]["z_sb"] = z_sb

            def stageB(k):
                """z2 production: rank2 -> exp(0.8 e) -> *D max D -> *mask"""
                ev_rhs, ev_lhs = st[k]["ev_rhs"], st[k]["ev_lhs"]
                dcol_sb, z_sb = st[k]["dcol_sb"], st[k]["z_sb"]
                EW = int(os.environ.get("K_EW", "1"))  # chunks per eadd tile
                ZPOOLN = int(os.environ.get("K_ZPN", "0"))  # z2 ts on Pool
                for ha in range(NCH // EW):
                    e_ps = eadd_ps.tile([128, EW, N], F32, tag="eadd")
                    for ci in range(EW):
                        c = EW * ha + ci
                        nc.tensor.matmul(
                            e_ps[:, ci, :],
                            ev_lhs[:, c * 128:(c + 1) * 128],
                            ev_rhs, start=True, stop=True)
                    nc.scalar.activation(
                        z_sb[:, EW * ha:EW * (ha + 1), :], e_ps,
                        AF.Exp, scale=0.8)
                    # z2 = D * max(u, 1) = max(u*D, D), in place (DVE 4x,
                    # last ZPOOLN chunks on Pool)
                    for ci in range(EW):
                        c = EW * ha + ci
                        e_z = nc.gpsimd if c >= NCH - ZPOOLN else nc.vector
                        e_z.tensor_scalar(
                            out=z_sb[:, c, :], in0=z_sb[:, c, :],
                            scalar1=dcol_sb[:, c:c + 1],
                            scalar2=dcol_sb[:, c:c + 1],
                            op0=ALU.mult, op1=ALU.max)
                # mask multiplicatively (DVE 2x), in place, in MSPLIT pieces
                # (last MPOOLN pieces on Pool)
                MPOOLN = int(os.environ.get("K_MPN", "0"))
                mw = NCH // MSPLIT
                for mi in range(MSPLIT):
                    sl = slice(mi * mw, (mi + 1) * mw)
                    e_m = nc.gpsimd if mi >= MSPLIT - MPOOLN else nc.vector
                    e_m.tensor_tensor(
                        out=z_sb[:, sl, :], in0=z_sb[:, sl, :],
                        in1=cap_sb[:, sl, :], op=ALU.mult)
                st[k]["z2_sb"] = z_sb

            def stage2(k):
                h_sb, z2_sb = st[k]["h_sb"], st[k]["z2_sb"]
                s_ps = mm_ps.tile([1, N], F32, tag="mm")
                for c in range(NCH):
                    nc.tensor.matmul(s_ps, ones_sb, z2_sb[:, c, :],
                                     start=(c == 0), stop=(c == NCH - 1))
                uT_ps = ut_psp.tile([128, N], F32, tag="ut")
                for c in range(NCH):
                    nc.tensor.matmul(uT_ps, h_sb[:, c, :], z2_sb[:, c, :],
                                     start=(c == 0), stop=(c == NCH - 1))

                s_row = work.tile([1, N], F32, tag="s_row")
                if E_SROW == "act":
                    nc.scalar.copy(out=s_row, in_=s_ps)
                else:
                    nc.vector.tensor_copy(out=s_row, in_=s_ps)
                sc_ps = ut_psp.tile([128, NCH], F32, tag="ut")
                for c in range(NCH):
                    nc.tensor.transpose(
                        sc_ps[:, c:c + 1], s_row[0:1, c * 128:(c + 1) * 128],
                        id_sb[0:1, 0:1])
                uT_sb = work.tile([128, N], F16, tag="uT_sb")
                if E_UTCOPY == "act":
                    nc.scalar.copy(out=uT_sb, in_=uT_ps)
                else:
                    nc.vector.tensor_copy(out=uT_sb, in_=uT_ps)
                sc_sb = work.tile([128, NCH], F32, tag="sc_sb")
                nc.vector.tensor_copy(out=sc_sb, in_=sc_ps)
                r_cols = work.tile([128, NCH], F32, tag="r_cols")
                nc.vector.reciprocal_approx_fast(r_cols, sc_sb)
                st[k]["uT_sb"] = uT_sb
                st[k]["r_cols"] = r_cols

            def stage3(k):
                b, t = divmod(k, T)
                uT_sb, r_cols = st[k]["uT_sb"], st[k]["r_cols"]
                u_ps = mm_ps.tile([128, NCH, F], F16, tag="mm")
                for c in range(NCH):
                    nc.tensor.transpose(
                        u_ps[:, c, :], uT_sb[:, c * 128:(c + 1) * 128], idh_sb)

                # v = u * r; elu(v) = max(v, min(exp(v)-1, 0)).
                # v/t/e1/o stay f32: gpsimd (Pool) only handles fp32 and
                # the tail e1/max run there.
                v_sb = work.tile([128, NCH, F], F32, tag="v_sb")
                for c in range(NCH):
                    nc.vector.tensor_scalar(
                        out=v_sb[:, c, :], in0=u_ps[:, c, :],
                        scalar1=r_cols[:, c:c + 1], scalar2=None,
                        op0=ALU.mult)
                fine = k >= NBT - int(os.environ.get("K_FINE", "1"))
                t_sb = work.tile([128, NCH, F], F32, tag="t_sb")
                e1_sb = work.tile([128, NCH, F], F32, tag="e1_sb")
                o_sb = work.tile([128, NCH, F], F32, tag="o_sb")
                od = out_h[b, :, t, :].rearrange("(c p) f -> p c f", p=128)
                if fine:
                    # last iterations: per-half tail so the first half's
                    # store overlaps the second half's compute
                    FQ = int(os.environ.get("K_FQ", "2"))
                    for hh in range(FQ):
                        w = NCH // FQ
                        sl = slice(w * hh, w * hh + w)
                        nc.scalar.activation(
                            t_sb[:, sl, :], v_sb[:, sl, :], AF.Exp)
                        nc.vector.tensor_scalar(
                            out=e1_sb[:, sl, :], in0=t_sb[:, sl, :],
                            scalar1=-1.0, scalar2=0.0,
                            op0=ALU.add, op1=ALU.min)
                        nc.vector.tensor_tensor(
                            out=o_sb[:, sl, :], in0=v_sb[:, sl, :],
                            in1=e1_sb[:, sl, :], op=ALU.max)
                        nc.sync.dma_start(out=od[:, sl, :],
                                          in_=o_sb[:, sl, :])
                    st[k].clear()
                    return
                nc.scalar.activation(t_sb, v_sb, AF.Exp)
                eng(E_E1).tensor_scalar(
                    out=e1_sb, in0=t_sb, scalar1=-1.0, scalar2=0.0,
                    op0=ALU.add, op1=ALU.min)
                # Pool TT supports add/mult only -> elu as m + e1
                m_sb = work.tile([128, NCH, F], F32, tag="m_sb")
                eng(os.environ.get("K_M", "pool")).tensor_scalar(
                    out=m_sb, in0=v_sb, scalar1=0.0, scalar2=None,
                    op0=ALU.max)
                eng(E_OMAX).tensor_tensor(
                    out=o_sb, in0=m_sb, in1=e1_sb, op=ALU.add)
                nc.sync.dma_start(out=od, in_=o_sb)
                st[k].clear()

            # software-pipelined emission: 4 stages A,B,C,D with lags
            # (stage X at iteration k processes bt k - lag(X)); emission
            # order within an iteration is oldest-first by default so no
            # engine queue head-blocks on not-yet-ready work.
            L1 = int(os.environ.get("K_L1", "2"))  # A -> B lag
            L2 = int(os.environ.get("K_L2", "1"))  # B -> C lag
            L3 = int(os.environ.get("K_L3", "1"))  # C -> D lag
            ORDER = os.environ.get("K_ORDER", "dbca")
            DEPTH = L1 + L2 + L3

            def body(_iv=None, unroll=1):
                for k in range(NBT + DEPTH):
                    def sa():
                        if k < NBT:
                            stageA(k)
                    def sb():
                        if L1 <= k < NBT + L1:
                            stageB(k - L1)
                    def sc():
                        if L1 + L2 <= k < NBT + L1 + L2:
                            stage2(k - L1 - L2)
                    def sd():
                        if k >= DEPTH:
                            stage3(k - DEPTH)
                    for ch in ORDER:
                        {"a": sa, "b": sb, "c": sc, "d": sd}[ch]()

            if reps == 1:
                body()
            else:
                with tc.For_i(0, reps, 1) as _iv:
                    body(_iv)

    nc.finalize()
    return nc


def prepare(x, W, a_src, a_dst, adj):
    """Build the program + per-core input maps (shared by kernel() and bench)."""
    f16 = np.float16

    x = np.ascontiguousarray(x, dtype=np.float32)
    W = np.ascontiguousarray(W, dtype=np.float32)
    a_src = np.asarray(a_src, dtype=np.float32)
    a_dst = np.asarray(a_dst, dtype=np.float32)
    adj = np.asarray(adj)

    allowed = (adj > 0) | np.eye(N, dtype=bool)               # [i, j]
    m01 = np.where(allowed, 1.0, 0.0).astype(np.float32)      # [i, j]
    capt = np.ascontiguousarray(m01.T.reshape(NCH, 128, N)).astype(f16)
    ws = W @ a_src
    wd = W @ a_dst
    wsd = np.stack([wd, ws], axis=1).astype(f16)
    sel = np.array([[1.0, 0.0], [0.0, 1.0]], dtype=np.float32)
    ident = np.eye(128, dtype=np.float32)
    onescol = np.ones((128, 1), dtype=np.float16)

    nc = _build_program()

    in_maps = []
    for c in range(NCORES):
        in_maps.append({
            "x": np.ascontiguousarray(x[c * B_PER_CORE:(c + 1) * B_PER_CORE]).astype(f16),
            "wb": W.astype(f16), "wsd": wsd, "capt": capt, "sel": sel,
            "ident": ident, "identh": ident.astype(f16), "onescol": onescol,
        })
    return nc, in_maps


def kernel(x, W, a_src, a_dst, adj):
    from concourse.bass_utils import run_bass_kernel_spmd

    nc, in_maps = prepare(x, W, a_src, a_dst, adj)
    res = run_bass_kernel_spmd(nc, in_maps, list(range(NCORES)))
    out = np.concatenate([res.results[c]["out"] for c in range(NCORES)], axis=0)
    return out  # [B, N, T, F]


# revision 29
# speedup vs baseline: 1.0004x; 1.0004x over previous
"""GAT layer kernel for Trainium2 (8 NeuronCores, SPMD data-parallel over B).

Reference computation (per (b,t) slice, N=512 nodes, D=F=128):
    h = x_bt @ W
    e[i,j] = leaky_relu(e_src[i] + e_dst[j], 0.2)
    e masked by adj|I, row-softmax, out = elu(alpha @ h)

v6 dataflow. Key identity: exp(leaky(e, 0.2)) = exp(0.2e) * max(exp(0.8e), 1)
 = C_i * D_j * max(u_ji, 1) with u = exp(0.8(es_i + ed_j)), C = exp(0.2 es),
D = exp(0.2 ed). The per-row factor C_i cancels in the softmax (alpha = z/s),
so the kernel aggregates z2_ji = D_j * max(u_ji, 1) * m_ij instead of the
full exp(leaky): ONE big activation pass (exp, scale=0.8) replaces the
baseline's Prelu+Exp pair, and the adjacency mask is applied
multiplicatively (no PE mask-prefill matmul). 2-byte tensors are fp16
(u <= exp(8.8) ~ 6.6e3 fits; fp16 mantissa keeps rel err ~7e-4).

Per (b,t), in eT = e^T [j, i] orientation so aggregation runs as
PSUM-accumulated matmuls with j as the contraction dim. Four software-
pipelined stages (A prep, B z2 production, C aggregation, D tail) with
lags (2,1,1), emitted oldest-first ("dbca") so no in-order engine queue
head-blocks on not-yet-ready work:
  A: ev rows [ed; es] = [wd|ws].T @ xT -> f32 PSUM; DVE evac to f16;
     ev_lhs=[ed;1], ev_rhs=[1;es] via Pool tensor_scalar (sel ptrs);
     D_j = exp(0.2 ed): 4 PE column-transposes (4B-aligned f16 slots)
     + one tiny ACT exp; h = xT @ W -> f32 PSUM -> DVE evac f16
  B: rank-2 matmul per chunk -> f32 PSUM (ring3); ACT exp(0.8 e) -> f16
     SBUF (the only big ACT pass); z2 = max(u*D, D) in-place DVE ts
     (4x mode, per-partition D ptr); mask: in-place DVE tt *m01 in
     halves (2x mode)
  C: s = ones.T @ z2, uT = h.T @ z2 (PSUM-accumulated); s_row via ACT
     copy, uT via ACT copy (f16); PE-transpose s to columns; DVE
     reciprocal_approx_fast [128,4]
  D: PE-transpose uT back to [i, f] f16 PSUM; v = u * r (DVE ts, f32);
     elu(v) = max(v,0) + min(exp(v)-1, 0): ACT exp, Pool e1/m ts,
     Pool add (gpsimd TT supports add/mult only, f32 only for TT)
"""

import numpy as np

B, N, T, D, F = 16, 512, 12, 128, 128
NCORES = 8
B_PER_CORE = B // NCORES
NCH = N // 128  # 4 chunks of 128 nodes


def _build_program(reps=1):
    import concourse.bacc as bacc
    import concourse.tile as tile
    from concourse import mybir

    import os
    F32 = mybir.dt.float32
    F16 = mybir.dt.float16
    AF = mybir.ActivationFunctionType
    ALU = mybir.AluOpType

    nc = bacc.Bacc()

    def eng(name):
        return {"pool": nc.gpsimd, "vector": nc.vector, "act": nc.scalar}[name]

    # engine assignment knobs
    E_HCOPY = os.environ.get("K_HCOPY", "vector")   # h evac: vector|act
    E_UTCOPY = os.environ.get("K_UTCOPY", "act")    # uT evac: vector|act
    E_SROW = os.environ.get("K_SROW", "act")        # s_row evac: vector|act
    E_FIX = os.environ.get("K_FIX", "pool")         # ev fixups: pool|vector
    E_E1 = os.environ.get("K_E1", "pool")         # tail e1: pool|vector
    E_OMAX = os.environ.get("K_OMAX", "pool")       # tail o=max: pool|vector
    MSPLIT = int(os.environ.get("K_MSPLIT", "2"))   # z2 mask-mult pieces

    x_h = nc.declare_dram_parameter("x", [B_PER_CORE, N, T, D], F16, isOutput=False)
    wb_h = nc.declare_dram_parameter("wb", [D, F], F16, isOutput=False)
    wsd_h = nc.declare_dram_parameter("wsd", [D, 2], F16, isOutput=False)
    capt_h = nc.declare_dram_parameter("capt", [NCH, 128, N], F16, isOutput=False)
    sel_h = nc.declare_dram_parameter("sel", [2, 2], F32, isOutput=False)
    ident_h = nc.declare_dram_parameter("ident", [128, 128], F32, isOutput=False)
    identh_h = nc.declare_dram_parameter("identh", [128, 128], F16, isOutput=False)
    ones_h = nc.declare_dram_parameter("onescol", [128, 1], F16, isOutput=False)
    out_h = nc.declare_dram_parameter("out", [B_PER_CORE, N, T, F], F32, isOutput=True)

    NBT = B_PER_CORE * T

    with tile.TileContext(nc) as tc:
        with (
            tc.tile_pool(name="consts", bufs=1) as consts,
            tc.tile_pool(name="xbuf", bufs=1) as xbuf,
            tc.tile_pool(name="work", bufs=int(os.environ.get("K_WORK", "6"))) as work,
            tc.tile_pool(name="zpool", bufs=int(os.environ.get("K_BIG", "5"))) as zpool,
            tc.tile_pool(name="hpool", bufs=int(os.environ.get("K_HP", "5"))) as hpool,
            # PSUM (16KB/partition = 8 banks of 2KB):
            #   mm ring3 x 2KB = 3, eadd (f32 quarters, 2KB) ring3 = 3,
            #   ut (f32 [128,512]) ring2 = 2  -> 8 banks
            tc.tile_pool(name="mm_ps", bufs=int(os.environ.get("K_MM", "3")), space="PSUM") as mm_ps,
            tc.tile_pool(name="eadd_ps", bufs=int(os.environ.get("K_EADD", "3")), space="PSUM") as eadd_ps,
            tc.tile_pool(name="ut_ps", bufs=int(os.environ.get("K_UT", "2")), space="PSUM") as ut_psp,
        ):
            wb_sb = consts.tile([D, F], F16)
            wsd_sb = consts.tile([D, 2], F16)
            cap_sb = consts.tile([128, NCH, N], F16)
            sel_sb = consts.tile([2, 2], F32)
            id_sb = consts.tile([128, 128], F32)
            idh_sb = consts.tile([128, 128], F16)
            ones_sb = consts.tile([128, 1], F16)

            # ---- transpose-DMA all of x: [n, d] slices land as [d, n] f16.
            XCH = int(os.environ.get("K_XCH", "4"))  # chunks per x-DMA
            XPRI = int(os.environ.get("K_XPRI", "2"))  # bts loaded pre-consts
            xT_all = xbuf.tile([128, NBT, N], F16, tag="xT")

            def load_x(k):
                b, t = divmod(k, T)
                for c0 in range(0, NCH, XCH):
                    c1 = min(c0 + XCH, NCH)
                    nc.sync.dma_start_transpose(
                        out=xT_all[:, k, c0 * 128:c1 * 128],
                        in_=x_h[b, c0 * 128:c1 * 128, t, :])

            # DMA order follows first use. x-loads go on the sync (SP)
            # queue; consts are issued in parallel from the Pool DGE queue
            # (idle at startup) so neither serializes the other. The big
            # cap_sb (mask) load goes last among early consts -- first use
            # is stageB's mask-mult, well after ev/dcol/h.
            cq = nc.sync
            if os.environ.get("K_DGE", "0") == "1":
                # consts issued from the (startup-idle) ACT/DVE DGE queues
                # so the SP queue only carries x-loads; everything lands
                # earlier and x(1..) isn't stuck behind the big cap issue.
                nc.scalar.dma_start(out=wsd_sb, in_=wsd_h[:, :])
                nc.gpsimd.dma_start(out=sel_sb, in_=sel_h[:, :])
                nc.gpsimd.dma_start(out=idh_sb, in_=identh_h[:, :])
                nc.scalar.dma_start(
                    out=cap_sb,
                    in_=capt_h[:, :, :].rearrange("c p i -> p c i"))
                nc.gpsimd.dma_start(out=wb_sb, in_=wb_h[:, :])
                nc.gpsimd.dma_start(out=ones_sb, in_=ones_h[:, :])
                nc.gpsimd.dma_start(out=id_sb, in_=ident_h[:, :])
                for k in range(NBT):
                    load_x(k)
            else:
                for k in range(min(XPRI, NBT)):
                    load_x(k)
                cq.dma_start(out=wsd_sb, in_=wsd_h[:, :])
                cq.dma_start(out=sel_sb, in_=sel_h[:, :])
                cq.dma_start(out=idh_sb, in_=identh_h[:, :])
                cq.dma_start(
                    out=cap_sb, in_=capt_h[:, :, :].rearrange("c p i -> p c i"))
                cq.dma_start(out=wb_sb, in_=wb_h[:, :])
                XDEF = int(os.environ.get("K_XDEF", "4"))
                for k in range(min(XPRI, NBT), min(XDEF, NBT)):
                    load_x(k)
                cq.dma_start(out=ones_sb, in_=ones_h[:, :])
                cq.dma_start(out=id_sb, in_=ident_h[:, :])
                for k in range(min(XDEF, NBT), NBT):
                    load_x(k)

            st = [dict() for _ in range(NBT)]

            def stageA(k):
                """prep: ev matmul+evac, D columns, h matmul+evac"""
                xT = xT_all[:, k, :]
                # ev rows [ed; es] (f32 PSUM), evac, fixups
                ev_ps = mm_ps.tile([2, N], F32, tag="mm")
                nc.tensor.matmul(ev_ps, wsd_sb, xT, start=True, stop=True)
                evb_sb = work.tile([2, N], F16, tag="evb")
                nc.vector.tensor_copy(out=evb_sb, in_=ev_ps)
                ev_rhs = work.tile([2, N], F16, tag="ev_rhs")
                ev_lhs = work.tile([2, N], F16, tag="ev_lhs")
                eng(E_FIX).tensor_scalar(
                    out=ev_lhs, in0=evb_sb, scalar1=sel_sb[:, 0:1],
                    scalar2=sel_sb[:, 1:2], op0=ALU.mult, op1=ALU.add)
                eng(E_FIX).tensor_scalar(
                    out=ev_rhs, in0=evb_sb, scalar1=sel_sb[:, 1:2],
                    scalar2=sel_sb[:, 0:1], op0=ALU.mult, op1=ALU.add)

                # D_j = exp(0.2 ed_j) as columns [128, NCH]. The f16
                # transpose outputs go to even column slots so each PSUM
                # write is 4-byte aligned (verifier requirement).
                dcol_ps = ut_psp.tile([128, 2 * NCH], F16, tag="ut")
                for c in range(NCH):
                    nc.tensor.transpose(
                        dcol_ps[:, 2 * c:2 * c + 1],
                        evb_sb[0:1, c * 128:(c + 1) * 128], idh_sb[0:1, 0:1])
                dcol_sb = work.tile([128, NCH], F32, tag="dcol")
                nc.scalar.activation(
                    dcol_sb, dcol_ps[:, 0:2 * NCH:2], AF.Exp, scale=0.2)

                # h projection -> f32 PSUM -> f16 SBUF
                h_ps = mm_ps.tile([128, NCH, F], F32, tag="mm")
                for c in range(NCH):
                    nc.tensor.matmul(
                        h_ps[:, c, :], xT[:, c * 128:(c + 1) * 128],
                        wb_sb, start=True, stop=True)
                h_sb = hpool.tile([128, NCH, F], F16, tag="h_sb")
                if E_HCOPY == "act":
                    nc.scalar.copy(out=h_sb, in_=h_ps)
                else:
                    nc.vector.tensor_copy(out=h_sb, in_=h_ps)
                st[k]["h_sb"] = h_sb
                st[k]["ev_rhs"] = ev_rhs
                st[k]["ev_lhs"] = ev_lhs
                st[k]["dcol_sb"] = dcol_sb
                z_sb = zpool.tile([128, NCH, N], F16, tag="z_sb")
                st[k]["z_sb"] = z_sb

            def stageB(k):
                """z2 production: rank2 -> exp(0.8 e) -> *D max D -> *mask"""
                ev_rhs, ev_lhs = st[k]["ev_rhs"], st[k]["ev_lhs"]
                dcol_sb, z_sb = st[k]["dcol_sb"], st[k]["z_sb"]
                EW = int(os.environ.get("K_EW", "1"))  # chunks per eadd tile
                ZPOOLN = int(os.environ.get("K_ZPN", "0"))  # z2 ts on Pool
                for ha in range(NCH // EW):
                    e_ps = eadd_ps.tile([128, EW, N], F32, tag="eadd")
                    for ci in range(EW):
                        c = EW * ha + ci
                        nc.tensor.matmul(
                            e_ps[:, ci, :],
                            ev_lhs[:, c * 128:(c + 1) * 128],
                            ev_rhs, start=True, stop=True)
                    nc.scalar.activation(
                        z_sb[:, EW * ha:EW * (ha + 1), :], e_ps,
                        AF.Exp, scale=0.8)
                    # z2 = D * max(u, 1) = max(u*D, D), in place (DVE 4x,
                    # last ZPOOLN chunks on Pool)
                    for ci in range(EW):
                        c = EW * ha + ci
                        e_z = nc.gpsimd if c >= NCH - ZPOOLN else nc.vector
                        e_z.tensor_scalar(
                            out=z_sb[:, c, :], in0=z_sb[:, c, :],
                            scalar1=dcol_sb[:, c:c + 1],
                            scalar2=dcol_sb[:, c:c + 1],
                            op0=ALU.mult, op1=ALU.max)
                # mask multiplicatively (DVE 2x), in place, in MSPLIT pieces
                # (last MPOOLN pieces on Pool)
                MPOOLN = int(os.environ.get("K_MPN", "0"))
                mw = NCH // MSPLIT
                for mi in range(MSPLIT):
                    sl = slice(mi * mw, (mi + 1) * mw)
                    e_m = nc.gpsimd if mi >= MSPLIT - MPOOLN else nc.vector
                    e_m.tensor_tensor(
                        out=z_sb[:, sl, :], in0=z_sb[:, sl, :],
                        in1=cap_sb[:, sl, :], op=ALU.mult)
                st[k]["z2_sb"] = z_sb

            def stage2(k):
                h_sb, z2_sb = st[k]["h_sb"], st[k]["z2_sb"]
                s_ps = mm_ps.tile([1, N], F32, tag="mm")
                for c in range(NCH):
                    nc.tensor.matmul(s_ps, ones_sb, z2_sb[:, c, :],
                                     start=(c == 0), stop=(c == NCH - 1))
                uT_ps = ut_psp.tile([128, N], F32, tag="ut")
                for c in range(NCH):
                    nc.tensor.matmul(uT_ps, h_sb[:, c, :], z2_sb[:, c, :],
                                     start=(c == 0), stop=(c == NCH - 1))

                s_row = work.tile([1, N], F32, tag="s_row")
                if E_SROW == "act":
                    nc.scalar.copy(out=s_row, in_=s_ps)
                else:
                    nc.vector.tensor_copy(out=s_row, in_=s_ps)
                sc_ps = ut_psp.tile([128, NCH], F32, tag="ut")
                for c in range(NCH):
                    nc.tensor.transpose(
                        sc_ps[:, c:c + 1], s_row[0:1, c * 128:(c + 1) * 128],
                        id_sb[0:1, 0:1])
                uT_sb = work.tile([128, N], F16, tag="uT_sb")
                if E_UTCOPY == "act":
                    nc.scalar.copy(out=uT_sb, in_=uT_ps)
                else:
                    nc.vector.tensor_copy(out=uT_sb, in_=uT_ps)
                sc_sb = work.tile([128, NCH], F32, tag="sc_sb")
                nc.vector.tensor_copy(out=sc_sb, in_=sc_ps)
                r_cols = work.tile([128, NCH], F32, tag="r_cols")
                nc.vector.reciprocal_approx_fast(r_cols, sc_sb)
                st[k]["uT_sb"] = uT_sb
                st[k]["r_cols"] = r_cols

            def stage3(k):
                b, t = divmod(k, T)
                uT_sb, r_cols = st[k]["uT_sb"], st[k]["r_cols"]
                u_ps = mm_ps.tile([128, NCH, F], F16, tag="mm")
                for c in range(NCH):
                    nc.tensor.transpose(
                        u_ps[:, c, :], uT_sb[:, c * 128:(c + 1) * 128], idh_sb)

                # v = u * r; elu(v) = max(v, min(exp(v)-1, 0)).
                # v/t/e1/o stay f32: gpsimd (Pool) only handles fp32 and
                # the tail e1/max run there.
                v_sb = work.tile([128, NCH, F], F32, tag="v_sb")
                for c in range(NCH):
                    nc.vector.tensor_scalar(
                        out=v_sb[:, c, :], in0=u_ps[:, c, :],
                        scalar1=r_cols[:, c:c + 1], scalar2=None,
                        op0=ALU.mult)
                fine = k >= NBT - int(os.environ.get("K_FINE", "1"))
                t_sb = work.tile([128, NCH, F], F32, tag="t_sb")
                e1_sb = work.tile([128, NCH, F], F32, tag="e1_sb")
                o_sb = work.tile([128, NCH, F], F32, tag="o_sb")
                od = out_h[b, :, t, :].rearrange("(c p) f -> p c f", p=128)
                if fine:
                    # last iterations: per-half tail so the first half's
                    # store overlaps the second half's compute
                    FQ = int(os.environ.get("K_FQ", "2"))
                    for hh in range(FQ):
                        w = NCH // FQ
                        sl = slice(w * hh, w * hh + w)
                        nc.scalar.activation(
                            t_sb[:, sl, :], v_sb[:, sl, :], AF.Exp)
                        nc.vector.tensor_scalar(
                            out=e1_sb[:, sl, :], in0=t_sb[:, sl, :],
                            scalar1=-1.0, scalar2=0.0,
                            op0=ALU.add, op1=ALU.min)
                        nc.vector.tensor_tensor(
                            out=o_sb[:, sl, :], in0=v_sb[:, sl, :],
                            in1=e1_sb[:, sl, :], op=ALU.max)
                        nc.sync.dma_start(out=od[:, sl, :],
                                          in_=o_sb[:, sl, :])
                    st[k].clear()
                    return
                nc.scalar.activation(t_sb, v_sb, AF.Exp)
                eng(E_E1).tensor_scalar(
                    out=e1_sb, in0=t_sb, scalar1=-1.0, scalar2=0.0,
                    op0=ALU.add, op1=ALU.min)
                # Pool TT supports add/mult only -> elu as m + e1
                m_sb = work.tile([128, NCH, F], F32, tag="m_sb")
                eng(os.environ.get("K_M", "pool")).tensor_scalar(
                    out=m_sb, in0=v_sb, scalar1=0.0, scalar2=None,
                    op0=ALU.max)
                eng(E_OMAX).tensor_tensor(
                    out=o_sb, in0=m_sb, in1=e1_sb, op=ALU.add)
                nc.sync.dma_start(out=od, in_=o_sb)
                st[k].clear()

            # software-pipelined emission: 4 stages A,B,C,D with lags
            # (stage X at iteration k processes bt k - lag(X)); emission
            # order within an iteration is oldest-first by default so no
            # engine queue head-blocks on not-yet-ready work.
            L1 = int(os.environ.get("K_L1", "2"))  # A -> B lag
            L2 = int(os.environ.get("K_L2", "1"))  # B -> C lag
            L3 = int(os.environ.get("K_L3", "1"))  # C -> D lag
            ORDER = os.environ.get("K_ORDER", "dbca")
            DEPTH = L1 + L2 + L3

            def body(_iv=None, unroll=1):
                for k in range(NBT + DEPTH):
                    def sa():
                        if k < NBT:
                            stageA(k)
                    def sb():
                        if L1 <= k < NBT + L1:
                            stageB(k - L1)
                    def sc():
                        if L1 + L2 <= k < NBT + L1 + L2:
                            stage2(k - L1 - L2)
                    def sd():
                        if k >= DEPTH:
                            stage3(k - DEPTH)
                    for ch in ORDER:
                        {"a": sa, "b": sb, "c": sc, "d": sd}[ch]()

            if reps == 1:
                body()
            else:
                with tc.For_i(0, reps, 1) as _iv:
                    body(_iv)

    nc.finalize()
    return nc


def prepare(x, W, a_src, a_dst, adj):
    """Build the program + per-core input maps (shared by kernel() and bench)."""
    f16 = np.float16

    x = np.ascontiguousarray(x, dtype=np.float32)
    W = np.ascontiguousarray(W, dtype=np.float32)
    a_src = np.asarray(a_src, dtype=np.float32)
    a_dst = np.asarray(a_dst, dtype=np.float32)
    adj = np.asarray(adj)

    allowed = (adj > 0) | np.eye(N, dtype=bool)               # [i, j]
    m01 = np.where(allowed, 1.0, 0.0).astype(np.float32)      # [i, j]
    capt = np.ascontiguousarray(m01.T.reshape(NCH, 128, N)).astype(f16)
    ws = W @ a_src
    wd = W @ a_dst
    wsd = np.stack([wd, ws], axis=1).astype(f16)
    sel = np.array([[1.0, 0.0], [0.0, 1.0]], dtype=np.float32)
    ident = np.eye(128, dtype=np.float32)
    onescol = np.ones((128, 1), dtype=np.float16)

    nc = _build_program()

    in_maps = []
    for c in range(NCORES):
        in_maps.append({
            "x": np.ascontiguousarray(x[c * B_PER_CORE:(c + 1) * B_PER_CORE]).astype(f16),
            "wb": W.astype(f16), "wsd": wsd, "capt": capt, "sel": sel,
            "ident": ident, "identh": ident.astype(f16), "onescol": onescol,
        })
    return nc, in_maps


def kernel(x, W, a_src, a_dst, adj):
    from concourse.bass_utils import run_bass_kernel_spmd

    nc, in_maps = prepare(x, W, a_src, a_dst, adj)
    res = run_bass_kernel_spmd(nc, in_maps, list(range(NCORES)))
    out = np.concatenate([res.results[c]["out"] for c in range(NCORES)], axis=0)
    return out  # [B, N, T, F]
